# revision 25
# baseline (speedup 1.0000x reference)
from contextlib import ExitStack

import numpy as np

import concourse.bass as bass
import concourse.mybir as mybir
from concourse.tile import TileContext
from concourse.bass_utils import run_bass_kernel_spmd
from concourse import bacc

f32 = mybir.dt.float32
f32r = mybir.dt.float32r
FT = mybir.ActivationFunctionType
OP = mybir.AluOpType

B = 8
D = 1024
NSEQ = 1024
H = 16
DH = 64
KT = 8
EPS = 1e-5
SCALE = float(1.0 / np.sqrt(np.float32(DH)))

N_CORES = 8


_ROUND_ENGINES = ("gpsimd", "vector", "scalar")


def _load_round(nc, pool, stg_pool, name, src, cols=None):
    c0, c1 = (0, src.shape[1]) if cols is None else cols
    w = c1 - c0
    tiles = []
    for t in range(KT):
        stg = stg_pool.tile([128, w], f32, name=f"stg_{name}_{t}", tag="stg")
        nc.sync.dma_start(out=stg, in_=src[t * 128:(t + 1) * 128, c0:c1])
        til = pool.tile([128, w], f32r, name=f"{name}_{t}", tag=f"t{t}")
        eng = _ROUND_ENGINES[t % 3]
        if eng == "gpsimd":
            nc.gpsimd.tensor_copy(til, stg)
        elif eng == "vector":
            nc.vector.tensor_copy(til, stg)
        else:
            nc.scalar.copy(til, stg)
        tiles.append(til)
    return tiles


def _body(nc, tc, io, rep, upto="E"):
    (qT, kT_i, rT, qres, wqT, wkT, wvT, woT, bqv, bkv, bv,
     g1, b1, g2, b2, out) = io
    es = ExitStack()
    with es:
        perm = es.enter_context(tc.tile_pool(name=f"perm{rep}", bufs=1))
        pp = es.enter_context(tc.tile_pool(name=f"pp{rep}", bufs=1,
                                           space="PSUM"))
        stg = es.enter_context(tc.tile_pool(name=f"stg{rep}", bufs=3))

        bq_sb = perm.tile([128, KT], f32)
        nc.sync.dma_start(out=bq_sb, in_=bqv[:, :])
        bk_sb = perm.tile([128, KT], f32)
        nc.sync.dma_start(out=bk_sb, in_=bkv[:, :])

        def bcast_1d(pool, vec, nm):
            t = pool.tile([128, D], f32, name=nm)
            ap = bass.AP(tensor=vec, offset=0, ap=[[0, 128], [1, D]])
            nc.sync.dma_start(out=t, in_=ap)
            return t

        bv_bc = bcast_1d(perm, bv, "bv_bc")

        eps_t = perm.tile([128, 1], f32)
        nc.vector.memset(eps_t, EPS)
        ones16 = perm.tile([128, H], f32)
        nc.vector.memset(ones16, 1.0)

        oT_pool = es.enter_context(tc.tile_pool(name=f"oT{rep}", bufs=1))
        oT = [oT_pool.tile([128, NSEQ], f32r, name=f"oT_{t}")
              for t in range(KT)]

        qkv_es = ExitStack()
        qk_pool = qkv_es.enter_context(tc.tile_pool(name=f"qk{rep}", bufs=1))
        qhT = [qk_pool.tile([128, NSEQ], f32r, name=f"qhT_{t}")
               for t in range(KT)]
        khT = [qk_pool.tile([128, NSEQ], f32r, name=f"khT_{t}")
               for t in range(KT)]

        for (w_src, x_src, b_sb, dst, wn, xn) in (
            (wqT, qT, bq_sb, qhT, "wq", "q"),
            (wkT, kT_i, bk_sb, khT, "wk", "k"),
        ):
            with (
                tc.tile_pool(name=f"w{wn}{rep}", bufs=1) as wpool,
                tc.tile_pool(name=f"x{xn}{rep}", bufs=1) as xpool,
            ):
                w_r = _load_round(nc, wpool, stg, wn, w_src)
                x_r = _load_round(nc, xpool, stg, xn, x_src)
                for dt in range(KT):
                    for nh in range(2):
                        ps = pp.tile([128, 512], f32, name=f"ps_{dt}_{nh}",
                                     tag="ps", bufs=2)
                        for kt in range(KT):
                            nc.tensor.matmul(
                                ps,
                                w_r[kt][:, dt * 128:(dt + 1) * 128],
                                x_r[kt][:, nh * 512:(nh + 1) * 512],
                                start=(kt == 0), stop=(kt == KT - 1),
                            )
                        nc.vector.tensor_scalar_add(
                            dst[dt][:, nh * 512:(nh + 1) * 512], ps,
                            b_sb[:, dt:dt + 1],
                        )

        if upto == "B":
            for t in range(KT):
                nc.sync.dma_start(out=out[t * 128:(t + 1) * 128, :],
                                  in_=qhT[t].bitcast(f32))
            qkv_es.close()
            return

        vext_pool = qkv_es.enter_context(tc.tile_pool(name=f"vx{rep}",
                                                      bufs=1))
        v_ext = [vext_pool.tile([128, H * (DH + 1)], f32r, name=f"vext_{t}")
                 for t in range(KT)]
        with (
            tc.tile_pool(name=f"wv{rep}", bufs=1) as wpool,
            tc.tile_pool(name=f"rb{rep}", bufs=1) as rpool,
        ):
            wv_r = _load_round(nc, wpool, stg, "wv", wvT)
            for rh in range(2):
                r_half = _load_round(nc, rpool, stg, f"r{rh}", rT,
                                     cols=(rh * 512, (rh + 1) * 512))
                for nt in range(4 * rh, 4 * rh + 4):
                    ncol = (nt - 4 * rh) * 128
                    ones_view = v_ext[nt].rearrange("p (h c) -> p h c",
                                                    c=DH + 1)
                    nc.vector.tensor_copy(ones_view[:, :, DH:DH + 1], ones16)
                    for dh2 in range(2):
                        ps = pp.tile([128, 512], f32,
                                     name=f"psv_{nt}_{dh2}",
                                     tag="ps", bufs=2)
                        for kt in range(KT):
                            nc.tensor.matmul(
                                ps,
                                r_half[kt][:, ncol:ncol + 128],
                                wv_r[kt][:, dh2 * 512:(dh2 + 1) * 512],
                                start=(kt == 0), stop=(kt == KT - 1),
                            )
                        dst_view = ones_view[:, dh2 * 8:(dh2 + 1) * 8, 0:DH]
                        nc.vector.tensor_add(
                            dst_view,
                            ps.rearrange("p (h c) -> p h c", c=DH),
                            bv_bc[:, dh2 * 512:(dh2 + 1) * 512]
                            .rearrange("p (h c) -> p h c", c=DH),
                        )

        if upto == "C":
            for t in range(KT):
                nc.sync.dma_start(out=out[t * 128:(t + 1) * 128, :],
                                  in_=v_ext[t][:, 0:1024].bitcast(f32))
            qkv_es.close()
            return

        with (
            tc.tile_pool(name=f"pT{rep}", bufs=9) as ppool,
            tc.tile_pool(name=f"att{rep}", bufs=4) as apool,
        ):
            for hp in range(H // 2):
                dt = hp
                p_tiles = {0: [], 1: []}
                for jt in range(KT):
                    jcol = jt * 128
                    for par in range(2):
                        row0 = par * DH
                        sp = pp.tile([128, 1024], f32,
                                     name=f"sps_{hp}_{jt}_{par}",
                                     tag="ps2", bufs=3)
                        for ih in range(2):
                            nc.tensor.matmul(
                                sp[:, ih * 512:(ih + 1) * 512],
                                khT[dt][row0:row0 + DH, jcol:jcol + 128],
                                qhT[dt][row0:row0 + DH,
                                        ih * 512:(ih + 1) * 512],
                                start=True, stop=True,
                            )
                        p_t = ppool.tile([128, 1024], f32r,
                                         name=f"pT_{hp}_{jt}_{par}",
                                         tag="pT")
                        nc.scalar.activation(p_t, sp, FT.Exp, scale=SCALE)
                        p_tiles[par].append(p_t)
                for par in range(2):
                    h = 2 * hp + par
                    row0 = par * DH
                    for ih in range(2):
                        icol = ih * 512
                        o_ps = pp.tile([DH + 1, 512], f32,
                                       name=f"o_{hp}_{ih}_{par}", tag="ps",
                                       bufs=2)
                        for jt in range(KT):
                            nc.tensor.matmul(
                                o_ps,
                                v_ext[jt][:, h * (DH + 1):(h + 1) * (DH + 1)],
                                p_tiles[par][jt][:, icol:icol + 512],
                                start=(jt == 0), stop=(jt == KT - 1),
                            )
                        ou = apool.tile([DH + 1, 512], f32,
                                        name=f"ou_{hp}_{ih}_{par}", tag="ou")
                        nc.vector.tensor_copy(ou[0:DH, :], o_ps[0:DH, :])
                        recip = apool.tile([1, 512], f32,
                                           name=f"rc_{hp}_{ih}_{par}",
                                           tag="rc")
                        nc.vector.reciprocal(recip, o_ps[DH:DH + 1, :])
                        bcast = apool.tile([DH, 512], f32,
                                           name=f"bb_{hp}_{ih}_{par}",
                                           tag="bb")
                        nc.gpsimd.partition_broadcast(bcast, recip, DH)
                        nc.vector.tensor_mul(
                            oT[dt][row0:row0 + DH, icol:icol + 512],
                            ou[0:DH, :], bcast)

        qkv_es.close()

        if upto in ("D",):
            for t in range(KT):
                nc.sync.dma_start(out=out[t * 128:(t + 1) * 128, :],
                                  in_=oT[t].bitcast(f32))
            return

        with (
            tc.tile_pool(name=f"wo{rep}", bufs=1) as wpool,
            tc.tile_pool(name=f"gb{rep}", bufs=1) as gbp,
            tc.tile_pool(name=f"ln{rep}", bufs=3) as lnp,
        ):
            g1_bc = bcast_1d(gbp, g1, "g1_bc")
            b1_bc = bcast_1d(gbp, b1, "b1_bc")
            g2_bc = bcast_1d(gbp, g2, "g2_bc")
            b2_bc = bcast_1d(gbp, b2, "b2_bc")
            wo_r = _load_round(nc, wpool, stg, "wo", woT)

            def layer_norm(x_in, xsum, g_bc, b_bc, out_ap, nm,
                           badd_engine="vector"):
                sq = lnp.tile([128, D], f32, name=f"sq_{nm}", tag="sq")
                sumsq = lnp.tile([128, 1], f32, name=f"ss_{nm}", tag="ss")
                nc.scalar.activation(sq, x_in, FT.Square, accum_out=sumsq)
                mean = lnp.tile([128, 1], f32, name=f"mn_{nm}", tag="mn")
                nc.vector.tensor_scalar_mul(mean, xsum, 1.0 / D)
                m2 = lnp.tile([128, 1], f32, name=f"m2_{nm}", tag="m2")
                nc.vector.tensor_scalar(m2, xsum, xsum, 1.0 / (D * D),
                                        op0=OP.mult, op1=OP.mult)
                var = lnp.tile([128, 1], f32, name=f"vr_{nm}", tag="vr")
                nc.vector.scalar_tensor_tensor(var, sumsq, 1.0 / D, m2,
                                               op0=OP.mult, op1=OP.subtract)
                std = lnp.tile([128, 1], f32, name=f"sd_{nm}", tag="sd")
                nc.scalar.activation(std, var, FT.Sqrt, bias=eps_t, scale=1.0)
                rstd = lnp.tile([128, 1], f32, name=f"rs_{nm}", tag="rs")
                nc.vector.reciprocal(rstd, std)
                xh = lnp.tile([128, D], f32, name=f"xh_{nm}", tag="xh")
                nc.vector.tensor_scalar(xh, x_in, mean, rstd,
                                        op0=OP.subtract, op1=OP.mult)
                xg = lnp.tile([128, D], f32, name=f"xg_{nm}", tag="xg")
                nc.vector.tensor_mul(xg, xh, g_bc)
                if badd_engine == "vector":
                    nc.vector.tensor_add(out_ap, xg, b_bc)
                else:
                    nc.gpsimd.tensor_add(out_ap, xg, b_bc)

            for it in range(KT):
                x_t = lnp.tile([128, D], f32, name=f"x_{it}", tag="x")
                xs = lnp.tile([128, 2], f32, name=f"xs_{it}", tag="xs")
                qr = lnp.tile([128, D], f32, name=f"qr_{it}", tag="qr")
                nc.sync.dma_start(out=qr, in_=qres[it * 128:(it + 1) * 128, :])
                for dh2 in range(2):
                    ps = pp.tile([128, 512], f32, name=f"mha_{it}_{dh2}",
                                 tag="ps", bufs=2)
                    for dt in range(KT):
                        nc.tensor.matmul(
                            ps,
                            oT[dt][:, it * 128:(it + 1) * 128],
                            wo_r[dt][:, dh2 * 512:(dh2 + 1) * 512],
                            start=(dt == 0), stop=(dt == KT - 1),
                        )
                    nc.vector.scalar_tensor_tensor(
                        x_t[:, dh2 * 512:(dh2 + 1) * 512], ps, 0.0,
                        qr[:, dh2 * 512:(dh2 + 1) * 512],
                        op0=OP.add, op1=OP.add,
                        accum_out=xs[:, dh2:dh2 + 1])
                xsum = lnp.tile([128, 1], f32, name=f"xsum_{it}", tag="xsum")
                nc.vector.tensor_add(xsum, xs[:, 0:1], xs[:, 1:2])

                res = lnp.tile([128, D], f32, name=f"res_{it}", tag="res")
                layer_norm(x_t, xsum, g1_bc, b1_bc, res, f"a{it}", "vector")
                x2 = lnp.tile([128, D], f32, name=f"x2_{it}", tag="x2")
                x2s = lnp.tile([128, 1], f32, name=f"x2s_{it}", tag="x2s")
                nc.vector.scalar_tensor_tensor(
                    x2, res, 0.0, res, op0=OP.max, op1=OP.add, accum_out=x2s)
                y = lnp.tile([128, D], f32, name=f"y_{it}", tag="y")
                layer_norm(x2, x2s, g2_bc, b2_bc, y, f"b{it}", "gpsimd")
                nc.sync.dma_start(out=out[it * 128:(it + 1) * 128, :], in_=y)


def _build(nrep=1, upto="E"):
    nc = bacc.Bacc("TRN2", target_bir_lowering=False, debug=True)

    def inp(name, shape):
        return nc.declare_dram_parameter(name, list(shape), f32,
                                         isOutput=False)

    io = (
        inp("qT", (D, NSEQ)), inp("kT", (D, NSEQ)), inp("rT", (D, NSEQ)),
        inp("qres", (NSEQ, D)),
        inp("wqT", (D, D)), inp("wkT", (D, D)), inp("wvT", (D, D)),
        inp("woT", (D, D)),
        inp("bqv", (128, KT)), inp("bkv", (128, KT)), inp("bv", (D,)),
        inp("g1", (D,)), inp("b1", (D,)), inp("g2", (D,)), inp("b2", (D,)),
        nc.declare_dram_parameter("out", [NSEQ, D], f32, isOutput=True),
    )

    with TileContext(nc) as tc, \
            nc.allow_low_precision(reason="float32r matmuls"):
        if nrep == 1:
            _body(nc, tc, io, 0, upto=upto)
        else:
            with tc.For_i(0, nrep, 1) as _i:
                _body(nc, tc, io, 0, upto=upto)
    nc.finalize()
    return nc


_NC_CACHE = {}


def _get_nc(nrep=1):
    if nrep not in _NC_CACHE:
        _NC_CACHE[nrep] = _build(nrep)
    return _NC_CACHE[nrep]


def _make_in_maps(k, q, r, Wk, bk, Wq, bq, Wv, bv, Wo, bo, g1, b1, g2, b2):
    wqT = np.ascontiguousarray(Wq.T)
    wkT = np.ascontiguousarray(Wk.T)
    wvT = np.ascontiguousarray(Wv.T)
    woT = np.ascontiguousarray(Wo.T)
    bqv = np.ascontiguousarray(bq.reshape(KT, 128).T)
    bkv = np.ascontiguousarray(bk.reshape(KT, 128).T)
    in_maps = []
    for bidx in range(B):
        in_maps.append({
            "qT": np.ascontiguousarray(q[bidx].T),
            "kT": np.ascontiguousarray(k[bidx].T),
            "rT": np.ascontiguousarray(r[bidx].T),
            "qres": np.ascontiguousarray(q[bidx] + bo[None, :]),
            "wqT": wqT, "wkT": wkT, "wvT": wvT, "woT": woT,
            "bqv": bqv, "bkv": bkv, "bv": bv,
            "g1": g1, "b1": b1, "g2": g2, "b2": b2,
        })
    return in_maps


def kernel(k, q, r, Wk, bk, Wq, bq, Wv, bv, Wo, bo, g1, b1, g2, b2):
    k = np.asarray(k, np.float32)
    q = np.asarray(q, np.float32)
    r = np.asarray(r, np.float32)
    in_maps = _make_in_maps(
        k, q, r,
        np.asarray(Wk, np.float32), np.asarray(bk, np.float32),
        np.asarray(Wq, np.float32), np.asarray(bq, np.float32),
        np.asarray(Wv, np.float32), np.asarray(bv, np.float32),
        np.asarray(Wo, np.float32), np.asarray(bo, np.float32),
        np.asarray(g1, np.float32), np.asarray(b1, np.float32),
        np.asarray(g2, np.float32), np.asarray(b2, np.float32))
    nc = _get_nc(1)
    res = run_bass_kernel_spmd(nc, in_maps, list(range(N_CORES)))
    return np.stack([res.results[i]["out"] for i in range(N_CORES)], axis=0)
